# revision 1
# baseline (speedup 1.0000x reference)
"""Trainium2 Bass kernel for nn_GTN_72679436583060 (GTN message passing).

Math: with w-softmax over a singleton axis each GTConv is exactly 2*A, so

    out = 2 * rownorm(4*A@A + I) @ A
        = diag(8 / (4*rowsum(M) + 1)) @ (M@A + 0.25*A)   with M = A@A

Sharding: row-wise over 8 cores, A replicated. Per core (rows R = 256):
  GEMM1 (transposed):  MT = A^T @ (A_rows^T)        (2048 x 256), lhsT = A tiles
  deg:                 rowsum(M) via a ones-column matmul sharing GEMM2's lhsT
  GEMM2:               P = M @ A + 0.25*A_rows       (256 x 2048), lhsT = MT tiles
  epilogue:            out = P * (8 / (4*deg + 1))   per-row scale

All matmuls in bf16 (1 cycle/row on PE), fp32 PSUM accumulation, fp32 output.
GEMM1 runs k-outer so the PE tracks the streaming A DMA; all 16 output tile
groups fit in 8 PSUM banks via zero-writing "bank clear" matmuls (which also
warm up the PE HAM clock during the initial DMA window).
"""

import numpy as np

N = 2048
P = 128
NCORES = 8
R = N // NCORES        # 256 rows per core
KT = N // P            # 16 partition tiles
MT = R // P            # 2 row subtiles per core
FD = 512               # PSUM bank free dim (fp32)
NT2 = N // FD          # 4 GEMM2 n-tiles

_CACHE = {}


def _build_bass():
    from contextlib import ExitStack

    import concourse.bass as bass  # noqa: F401
    import concourse.mybir as mybir
    import concourse.tile as tile
    from concourse import bacc

    dt = mybir.dt
    fp32 = dt.float32
    bf16 = dt.bfloat16
    Alu = mybir.AluOpType

    nc = bacc.Bacc(None, target_bir_lowering=False)
    a_d = nc.dram_tensor("a", [N, N], bf16, kind="ExternalInput")
    art_d = nc.dram_tensor("art", [N, R], bf16, kind="ExternalInput")
    ar_d = nc.dram_tensor("ar", [R, N], bf16, kind="ExternalInput")
    ones_d = nc.dram_tensor("ones", [P, 1], bf16, kind="ExternalInput")
    iq_d = nc.dram_tensor("iq", [P, P], bf16, kind="ExternalInput")
    out_d = nc.dram_tensor("out", [R, N], fp32, kind="ExternalOutput")

    with tile.TileContext(nc) as tc, ExitStack() as ctx:
        a_pool = ctx.enter_context(tc.tile_pool(name="a", bufs=KT))
        art_pool = ctx.enter_context(tc.tile_pool(name="art", bufs=KT))
        ar_pool = ctx.enter_context(tc.tile_pool(name="ar", bufs=MT))
        mt_pool = ctx.enter_context(tc.tile_pool(name="mt", bufs=KT))
        const_pool = ctx.enter_context(tc.tile_pool(name="const", bufs=1))
        outsb_pool = ctx.enter_context(tc.tile_pool(name="outsb", bufs=4))
        sc_pool = ctx.enter_context(tc.tile_pool(name="sc", bufs=4))

        zeros_t = const_pool.tile([P, FD], bf16, tag="zeros")
        nc.vector.memset(zeros_t[:], 0.0)

        # Stream A row-tiles (and the matching ART tiles) in k order; they
        # stay resident: GEMM1 uses A as lhsT, GEMM2 reuses it as rhs.
        # The tiny const/ar loads are issued last — they are only needed in
        # GEMM2, and issuing them first would delay the first k-sweep.
        # The stream is HBM-bound (~330 GB/s aggregate); alternating the
        # big A tiles between the two HWDGE queues (sync/scalar) with
        # per-tile granularity keeps the k-sweep dependencies thin.
        a_tiles, art_tiles = [], []
        for k in range(KT):
            rt = art_pool.tile([P, R], bf16, tag="art")
            nc.sync.dma_start(rt[:], art_d[k * P:(k + 1) * P, :])
            art_tiles.append(rt)
            at = a_pool.tile([P, N], bf16, tag="a")
            eng = nc.sync if k % 2 == 0 else nc.scalar
            eng.dma_start(at[:], a_d[k * P:(k + 1) * P, :])
            a_tiles.append(at)
        ar_tiles = []
        for m in range(MT):
            t = ar_pool.tile([P, N], bf16, tag="ar")
            nc.sync.dma_start(t[:], ar_d[m * P:(m + 1) * P, :])
            ar_tiles.append(t)
        ones_t = const_pool.tile([P, 1], bf16, tag="ones")
        nc.sync.dma_start(ones_t[:], ones_d[:, :])
        iq_t = const_pool.tile([P, P], bf16, tag="iq")
        nc.sync.dma_start(iq_t[:], iq_d[:, :])

        # ---- GEMM1: MT[j, r] = sum_k A[k, j] * A_rows[r, k], k-outer ----
        # Two j-groups share each PSUM bank. A start=True zero matmul per
        # bank clears it and sets every has_written bit, so all real
        # matmuls accumulate with start=False regardless of issue order.
        mt_tiles = [None] * KT
        # One shared PSUM pool (8 banks, one tag) for GEMM1 pair tiles,
        # GEMM2 output tiles and deg tiles: GEMM2's first allocations reuse
        # slots as soon as individual pair tiles are copied out, instead of
        # stalling on a whole-pool release at the phase boundary.
        with tc.tile_pool(name="psum", bufs=8, space="PSUM") as psum_pool:
            # Per-bank zero matmul: start=True clears the whole bank; writing
            # [255:257) spans both half-bank groups, so WAW deps keep every
            # real matmul ordered after the clear. Elements outside [255:257)
            # keep has_written unset, so each group's first real matmul
            # overwrites (= accumulate onto zero).
            pairs = []
            for b in range(KT // 2):
                ps = psum_pool.tile([P, FD], fp32, tag="bank", name=f"pair_{b}")
                nc.tensor.matmul(
                    ps[:, R - 1:R + 1], zeros_t[:, 0:P], zeros_t[:, 0:2],
                    start=True, stop=False, skip_group_check=True,
                )
                pairs.append(ps)
            for k in range(KT):
                for j in range(KT):
                    half = (j % 2) * R
                    nc.tensor.matmul(
                        pairs[j // 2][:, half:half + R],
                        a_tiles[k][:, j * P:(j + 1) * P],
                        art_tiles[k][:],
                        start=False, stop=(k == KT - 1),
                        skip_group_check=True,
                    )
            for j in range(KT):
                half = (j % 2) * R
                mt = mt_pool.tile([P, R], bf16, tag="mt")
                nc.vector.tensor_copy(mt[:], pairs[j // 2][:, half:half + R])
                mt_tiles[j] = mt

            # ---- GEMM2 + deg + epilogue ----
            # The 0.25*I matmul doubles as each bank's accumulation-group
            # starter (start=True clears the bank and seeds it with
            # 0.25*A_rows), so banks finish at their last j matmul.
            # m=0 runs j-outer (tracks the mt copies with no stall);
            # m=1 runs n-outer so its four banks complete staggered and the
            # final epilogues pipeline with PE instead of bunching at the end.
            def emit_epilogue(m, n, psum_tile, sca):
                ot = outsb_pool.tile([P, FD], fp32, tag="ot",
                                     name=f"ot_{m}_{n}")
                nc.vector.tensor_scalar(
                    out=ot[:], in0=psum_tile[:], scalar1=sca[:],
                    scalar2=None, op0=Alu.mult,
                )
                eng = nc.sync if n % 2 == 0 else nc.scalar
                eng.dma_start(
                    out_d[m * P:(m + 1) * P, n * FD:(n + 1) * FD], ot[:]
                )

            def emit_deg_scale(m, deg_ps):
                # scale = 8 / (4*deg + 1) == 1 / (0.5*deg + 0.125)
                t1 = sc_pool.tile([P, 1], fp32, tag="t1", name=f"t1_{m}")
                nc.vector.tensor_scalar(
                    out=t1[:], in0=deg_ps[:], scalar1=0.5, scalar2=0.125,
                    op0=Alu.mult, op1=Alu.add,
                )
                sca = sc_pool.tile([P, 1], fp32, tag="sca", name=f"sca_{m}")
                nc.vector.reciprocal(sca[:], t1[:])
                return sca

            # m = 0: j-outer
            m = 0
            outs_ps = [psum_pool.tile([P, FD], fp32, tag="bank",
                                      name=f"outps0_{i}") for i in range(NT2)]
            deg_full = psum_pool.tile([P, FD], fp32, tag="bank", name="deg_0")
            deg_ps = deg_full[:, 0:1]
            for n in range(NT2):
                nc.tensor.matmul(
                    outs_ps[n][:], iq_t[:],
                    ar_tiles[m][:, n * FD:(n + 1) * FD],
                    start=True, stop=False,
                )
            for j in range(KT):
                lhsT = mt_tiles[j][:, m * P:(m + 1) * P]
                for n in range(NT2):
                    nc.tensor.matmul(
                        outs_ps[n][:], lhsT,
                        a_tiles[j][:, n * FD:(n + 1) * FD],
                        start=False, stop=(j == KT - 1),
                    )
                nc.tensor.matmul(
                    deg_ps[:], lhsT, ones_t[:],
                    start=(j == 0), stop=(j == KT - 1),
                )
            sca = emit_deg_scale(m, deg_ps)
            for n in range(NT2):
                emit_epilogue(m, n, outs_ps[n], sca)

            # m = 1: n-outer, deg rides along with the n=0 bank
            m = 1
            deg_full = psum_pool.tile([P, FD], fp32, tag="bank", name="deg_1")
            deg_ps = deg_full[:, 0:1]
            sca = None
            for n in range(NT2):
                ops = psum_pool.tile([P, FD], fp32, tag="bank",
                                     name=f"outps1_{n}")
                nc.tensor.matmul(
                    ops[:], iq_t[:], ar_tiles[m][:, n * FD:(n + 1) * FD],
                    start=True, stop=False,
                )
                for j in range(KT):
                    lhsT = mt_tiles[j][:, m * P:(m + 1) * P]
                    nc.tensor.matmul(
                        ops[:], lhsT, a_tiles[j][:, n * FD:(n + 1) * FD],
                        start=False, stop=(j == KT - 1),
                    )
                    if n == 0:
                        nc.tensor.matmul(
                            deg_ps[:], lhsT, ones_t[:],
                            start=(j == 0), stop=(j == KT - 1),
                        )
                if n == 0:
                    sca = emit_deg_scale(m, deg_ps)
                emit_epilogue(m, n, ops, sca)
    nc.compile()
    return nc


def _get_nc():
    if "nc" not in _CACHE:
        _CACHE["nc"] = _build_bass()
    return _CACHE["nc"]


def _make_in_maps(A_f32):
    import ml_dtypes

    bf = ml_dtypes.bfloat16
    Ab = A_f32.astype(bf)
    ATb = np.ascontiguousarray(Ab.T)

    ones = np.ones((P, 1), dtype=bf)
    iq = (0.25 * np.eye(P, dtype=np.float32)).astype(bf)
    in_maps = []
    for c in range(NCORES):
        sl = slice(c * R, (c + 1) * R)
        in_maps.append({
            "a": Ab,
            "art": np.ascontiguousarray(ATb[:, sl]),
            "ar": np.ascontiguousarray(Ab[sl, :]),
            "ones": ones,
            "iq": iq,
        })
    return in_maps


def kernel(A, w1a=None, w1b=None, w2a=None, **_unused):
    # w1a/w1b/w2a only enter the reference through a softmax over a
    # singleton axis (== 1.0), so the output does not depend on them.
    from concourse.bass_utils import run_bass_kernel_spmd

    A = np.asarray(A, dtype=np.float32)
    assert A.shape == (N, N), A.shape
    nc = _get_nc()
    in_maps = _make_in_maps(A)
    res = run_bass_kernel_spmd(nc, in_maps, core_ids=list(range(NCORES)))
    out = np.concatenate(
        [res.results[c]["out"] for c in range(NCORES)], axis=0
    )
    return out[None].astype(np.float32)



# revision 6
# speedup vs baseline: 1.2998x; 1.2998x over previous
"""Trainium2 Bass kernel for nn_GTN_72679436583060 (GTN message passing).

Math: with w-softmax over a singleton axis each GTConv is exactly 2*A, so

    out = 2 * rownorm(4*A@A + I) @ A
        = diag(8 / (4*d + 1)) @ (M@A + 0.25*A)   with M = A@A, d = rowsum(M)

The 0.25*A term is ~2.4e-7 of M@A in relative magnitude (M@A entries are
~5e5, A entries < 1) and is dropped.

Sharding: row-wise over 8 cores, A replicated. Per core (rows R = 256):
  GEMM1 (transposed):  MT = A^T @ (A_rows^T)    (2048 x 256), lhsT = A tiles
  requant:             MT8 = MT/64 cast fp8     (scalar+vector engine copies)
  GEMM2:               P' = (M/64) @ A          (256 x 2048), lhsT = MT8 tiles
  deg:                 d/64 = rowsum(M/64) via a ones-column matmul
  epilogue:            out = P' / (d/128 + 1/512)  per-row scale, bf16 out

All matmuls run fp8e4 in DoubleRow perf mode (2 k-subtiles per instruction,
0.5 PE cycles per output row = 4x bf16 FLOP rate), fp32 PSUM accumulation.
A is pre-interleaved on the host into k-pair layout [t, p, i, c] =
A[t*256 + i*128 + p, c] so every DMA is contiguous and every matmul operand
is a plain [128, 2, F] slice.  M/64 ~ 8 << 240 = fp8e4 max, A in [0,1);
host-validated end-to-end rel err of this scheme is ~1.6e-3 (gate 2e-2).
"""

import numpy as np

N = 2048
P = 128
NCORES = 8
R = N // NCORES        # 256 rows per core
KP = N // (2 * P)      # 8 k-pair tiles (256 rows each)
FD = 512               # PSUM bank free dim (fp32)
NT2 = N // FD          # 4 GEMM2 n-tiles

_CACHE = {}


def _build_bass():
    from contextlib import ExitStack

    import concourse.bass as bass  # noqa: F401
    import concourse.mybir as mybir
    import concourse.tile as tile
    from concourse import bacc

    dt = mybir.dt
    fp32 = dt.float32
    bf16 = dt.bfloat16
    fp8 = dt.float8e4
    Alu = mybir.AluOpType
    Act = mybir.ActivationFunctionType
    DR = mybir.MatmulPerfMode.DoubleRow

    nc = bacc.Bacc(None, target_bir_lowering=False)
    # a_il[t, p, i, c]  = A[t*256 + i*128 + p, c]
    a_d = nc.dram_tensor("a", [KP, P, 2, N], fp8, kind="ExternalInput")
    # art_il[t, p, i, r] = A[row0 + r, t*256 + i*128 + p]
    art_d = nc.dram_tensor("art", [KP, P, 2, R], fp8, kind="ExternalInput")
    out_d = nc.dram_tensor("out", [R, N], bf16, kind="ExternalOutput")

    with tile.TileContext(nc) as tc, ExitStack() as ctx:
        a_pool = ctx.enter_context(tc.tile_pool(name="a", bufs=KP))
        art_pool = ctx.enter_context(tc.tile_pool(name="art", bufs=KP))
        mt_pool = ctx.enter_context(tc.tile_pool(name="mt", bufs=KP))
        const_pool = ctx.enter_context(tc.tile_pool(name="const", bufs=1))
        outsb_pool = ctx.enter_context(tc.tile_pool(name="outsb", bufs=4))
        sc_pool = ctx.enter_context(tc.tile_pool(name="sc", bufs=4))

        ones_t = const_pool.tile([P, 2, 1], fp8, tag="ones")
        nc.vector.memset(ones_t[:], 1.0)

        # Stream A k-pair tiles (and the matching ART tiles) in t order;
        # they stay resident: GEMM1 uses A as lhsT, GEMM2 reuses it as rhs.
        # Alternate the big A tiles between the two HWDGE queues
        # (sync/scalar) so the stream keeps both DMA engines busy.
        a_tiles, art_tiles = [], []
        for t in range(KP):
            rt = art_pool.tile([P, 2, R], fp8, tag="art")
            nc.sync.dma_start(rt[:], art_d[t])
            art_tiles.append(rt)
            at = a_pool.tile([P, 2, N], fp8, tag="a")
            eng = nc.sync if t % 2 == 0 else nc.scalar
            eng.dma_start(at[:], a_d[t])
            a_tiles.append(at)

        # ---- GEMM1: MT[j*128+m, r] = sum_k A[k, j*128+m] * A[row0+r, k] ----
        # DoubleRow, t-outer so the PE tracks the streaming A DMA.  Each
        # PSUM bank holds one j-pair (two [128, 256] MT tiles = the exact
        # DoubleRow k-pair layout GEMM2's lhsT wants).
        with tc.tile_pool(name="psum", bufs=8, space="PSUM") as psum_pool:
            # Bank init rides on the t=0 matmuls: the half-0 matmul has
            # start=True, which marks the whole 2KB bank pending-zero; the
            # half-1 matmul (start=False, program-ordered after it on the
            # PE) writes into still-pending bytes and therefore also
            # overwrites instead of accumulating.
            pairs = [psum_pool.tile([P, 2, R], fp32, tag="bank",
                                    name=f"pair_{b}") for b in range(KP)]
            for t in range(KP):
                for j2 in range(KP):
                    for half in range(2):
                        j = 2 * j2 + half
                        nc.tensor.matmul(
                            pairs[j2][:, half, :],
                            a_tiles[t][:, :, j * P:(j + 1) * P],
                            art_tiles[t][:],
                            start=(t == 0 and half == 0),
                            stop=(t == KP - 1),
                            perf_mode=DR, skip_group_check=True,
                        )

            # Requantize MT -> fp8 (MT/64); alternate scalar/vector engines
            # so the copies drain two banks per GEMM2 j2-round.
            mt_tiles = []
            for j2 in range(KP):
                mt = mt_pool.tile([P, 2, R], fp8, tag="mt")
                if j2 % 2 == 0:
                    nc.scalar.activation(mt[:], pairs[j2][:], Act.Copy,
                                         scale=1.0 / 64.0)
                else:
                    nc.vector.tensor_scalar(
                        out=mt[:], in0=pairs[j2][:], scalar1=1.0 / 64.0,
                        scalar2=None, op0=Alu.mult,
                    )
                mt_tiles.append(mt)

            # ---- GEMM2 + deg + epilogue ----
            def emit_deg_scale(m, deg_ps):
                # psum deg = d/64;  scale = 1 / (d/128 + 1/512)
                t1 = sc_pool.tile([P, 1], fp32, tag="t1", name=f"t1_{m}")
                nc.vector.tensor_scalar(
                    out=t1[:], in0=deg_ps[:], scalar1=0.5,
                    scalar2=1.0 / 512.0, op0=Alu.mult, op1=Alu.add,
                )
                sca = sc_pool.tile([P, 1], fp32, tag="sca", name=f"sca_{m}")
                nc.vector.reciprocal(sca[:], t1[:])
                return sca

            def emit_epilogue(m, n, psum_tile, sca):
                ot = outsb_pool.tile([P, FD], bf16, tag="ot",
                                     name=f"ot_{m}_{n}")
                nc.vector.tensor_scalar(
                    out=ot[:], in0=psum_tile[:], scalar1=sca[:],
                    scalar2=None, op0=Alu.mult,
                )
                eng = nc.sync if n % 2 == 0 else nc.scalar
                eng.dma_start(
                    out_d[m * P:(m + 1) * P, n * FD:(n + 1) * FD], ot[:]
                )

            for m in range(2):
                outs_ps = [psum_pool.tile([P, FD], fp32, tag="bank",
                                          name=f"outps{m}_{i}")
                           for i in range(NT2)]
                deg_full = psum_pool.tile([P, FD], fp32, tag="bank",
                                          name=f"deg_{m}")
                deg_ps = deg_full[:, 0:1]
                # j2-outer: each j2 round needs only mt_tiles[j2], so the
                # PE starts as soon as the first requant copy lands.
                for j2 in range(KP):
                    lhsT = mt_tiles[j2][:, :, m * P:(m + 1) * P]
                    for n in range(NT2):
                        nc.tensor.matmul(
                            outs_ps[n][:], lhsT,
                            a_tiles[j2][:, :, n * FD:(n + 1) * FD],
                            start=(j2 == 0), stop=(j2 == KP - 1),
                            perf_mode=DR,
                        )
                    nc.tensor.matmul(
                        deg_ps[:], lhsT, ones_t[:],
                        start=(j2 == 0), stop=(j2 == KP - 1),
                        perf_mode=DR,
                    )
                sca = emit_deg_scale(m, deg_ps)
                for n in range(NT2):
                    emit_epilogue(m, n, outs_ps[n], sca)
    nc.compile()
    return nc


def _get_nc():
    if "nc" not in _CACHE:
        _CACHE["nc"] = _build_bass()
    return _CACHE["nc"]


def _make_in_maps(A_f32):
    import ml_dtypes

    f8 = ml_dtypes.float8_e4m3
    A8 = A_f32.astype(f8)
    # a_il[t, p, i, c] = A[t*256 + i*128 + p, c]
    a_il = np.ascontiguousarray(
        A8.reshape(KP, 2, P, N).transpose(0, 2, 1, 3)
    )
    AT8 = A8.T
    in_maps = []
    for c in range(NCORES):
        sl = slice(c * R, (c + 1) * R)
        # art_il[t, p, i, r] = A[row0 + r, t*256 + i*128 + p]
        art_il = np.ascontiguousarray(
            AT8[:, sl].reshape(KP, 2, P, R).transpose(0, 2, 1, 3)
        )
        in_maps.append({"a": a_il, "art": art_il})
    return in_maps


def kernel(A, w1a=None, w1b=None, w2a=None, **_unused):
    # w1a/w1b/w2a only enter the reference through a softmax over a
    # singleton axis (== 1.0), so the output does not depend on them.
    from concourse.bass_utils import run_bass_kernel_spmd

    A = np.asarray(A, dtype=np.float32)
    assert A.shape == (N, N), A.shape
    nc = _get_nc()
    in_maps = _make_in_maps(A)
    res = run_bass_kernel_spmd(nc, in_maps, core_ids=list(range(NCORES)))
    out = np.concatenate(
        [res.results[c]["out"] for c in range(NCORES)], axis=0
    )
    return out[None].astype(np.float32)


# revision 9
# speedup vs baseline: 1.6040x; 1.2341x over previous
"""Trainium2 Bass kernel for nn_GTN_72679436583060 (GTN message passing).

Math: with w-softmax over a singleton axis each GTConv is exactly 2*A, so

    out = 2 * rownorm(4*A@A + I) @ A
        = diag(8 / (4*d + 1)) @ (M@A + 0.25*A)   with M = A@A, d = rowsum(M)

The 0.25*A term is ~2.4e-7 of M@A in relative magnitude (M@A entries are
~5e5, A entries < 1) and is dropped.

Sharding: row-wise over 8 cores, A replicated. Per core (rows R = 256):
  GEMM1 (transposed):  MT = A^T @ (A_rows^T)    (2048 x 256), lhsT = A tiles
  requant:             MT8 = MT/64 cast fp8     (scalar/vector/gpsimd copies)
  GEMM2:               P' = (M/64) @ A          (256 x 2048), lhsT = MT8 tiles
  deg:                 d/64 = rowsum(M/64) via a ones-column matmul
  epilogue:            out = P' / (d/128 + 1/512)  per-row scale, bf16 out

All matmuls run fp8e4 DoubleRow (2 k-subtiles per instruction; measured on
HW this is 1 PE cycle per output row, i.e. 2x bf16 FLOP rate - the PE floor
for the two GEMMs is 65536 cycles/core).  Schedule notes, from traces:
  - The PE clock p-state needs ~3us of continuous work to reach full speed,
    so a run of warm-up matmuls on constant data fills the initial DMA
    window (the first real matmul otherwise runs the whole GEMM1 at half
    clock).
  - t=0's A tile is DMA'd in two column chunks so GEMM1 can start early.
  - GEMM2 runs n-outer so each PSUM bank completes after its own j2 sweep
    and the epilogue + output DMA pipeline behind the next bank's matmuls.
M/64 ~ 8 << 240 = fp8e4 max, A in [0,1); host-validated end-to-end rel err
of this scheme is ~1.6e-3 (gate 2e-2).
"""

import numpy as np

N = 2048
P = 128
NCORES = 8
R = N // NCORES        # 256 rows per core
KP = N // (2 * P)      # 8 k-pair tiles (256 rows each)
FD = 512               # PSUM bank free dim (fp32)
NT2 = N // FD          # 4 GEMM2 n-tiles
WARMUP = 12            # p-state warm-up matmuls (N=256 each)

_CACHE = {}


def _build_bass():
    from contextlib import ExitStack

    import concourse.bass as bass  # noqa: F401
    import concourse.mybir as mybir
    import concourse.tile as tile
    from concourse import bacc

    dt = mybir.dt
    fp32 = dt.float32
    bf16 = dt.bfloat16
    fp8 = dt.float8e4
    Alu = mybir.AluOpType
    Act = mybir.ActivationFunctionType
    DR = mybir.MatmulPerfMode.DoubleRow

    nc = bacc.Bacc(None, target_bir_lowering=False)
    # a_il[t, p, i, c]  = A[t*256 + i*128 + p, c]
    a_d = nc.dram_tensor("a", [KP, P, 2, N], fp8, kind="ExternalInput")
    # art_il[t, p, i, r] = A[row0 + r, t*256 + i*128 + p]
    art_d = nc.dram_tensor("art", [KP, P, 2, R], fp8, kind="ExternalInput")
    out_d = nc.dram_tensor("out", [R, N], bf16, kind="ExternalOutput")

    with tile.TileContext(nc) as tc, ExitStack() as ctx:
        a_pool = ctx.enter_context(tc.tile_pool(name="a", bufs=KP))
        art_pool = ctx.enter_context(tc.tile_pool(name="art", bufs=KP))
        mt_pool = ctx.enter_context(tc.tile_pool(name="mt", bufs=KP))
        const_pool = ctx.enter_context(tc.tile_pool(name="const", bufs=1))
        outsb_pool = ctx.enter_context(tc.tile_pool(name="outsb", bufs=4))
        sc_pool = ctx.enter_context(tc.tile_pool(name="sc", bufs=4))

        ones_t = const_pool.tile([P, 2, 1], fp8, tag="ones")
        nc.vector.memset(ones_t[:], 1.0)
        warm_t = const_pool.tile([P, 2, R], fp8, tag="warm")
        nc.vector.memset(warm_t[:], 1.0)

        # Stream the k-pair tiles; each t needs (art[t], a[t]) so the pairs
        # alternate between the two HWDGE queues (sync/scalar) to keep both
        # busy and make t=0/t=1 land in parallel.  t=0's A tile is split in
        # two column chunks so its first 8 j-columns arrive sooner.
        a_tiles = [a_pool.tile([P, 2, N], fp8, tag="a", name=f"a_{t}")
                   for t in range(KP)]
        art_tiles = [art_pool.tile([P, 2, R], fp8, tag="art",
                                   name=f"art_{t}") for t in range(KP)]
        nc.sync.dma_start(art_tiles[0][:], art_d[0])
        nc.sync.dma_start(a_tiles[0][:, :, 0:N // 2],
                          a_d[0][:, :, 0:N // 2])
        nc.scalar.dma_start(art_tiles[1][:], art_d[1])
        nc.scalar.dma_start(a_tiles[1][:], a_d[1])
        nc.sync.dma_start(a_tiles[0][:, :, N // 2:N],
                          a_d[0][:, :, N // 2:N])
        for t in range(2, KP):
            eng = nc.sync if t % 2 == 0 else nc.scalar
            eng.dma_start(art_tiles[t][:], art_d[t])
            eng.dma_start(a_tiles[t][:], a_d[t])

        # ---- GEMM1: MT[j*128+m, r] = sum_k A[k, j*128+m] * A[row0+r, k] ----
        # DoubleRow, t-outer so the PE tracks the streaming A DMA.  Each
        # PSUM bank holds one j-pair (two [128, 256] MT tiles = the exact
        # DoubleRow k-pair layout GEMM2's lhsT wants).
        with tc.tile_pool(name="psum", bufs=8, space="PSUM") as psum_pool:
            pairs = [psum_pool.tile([P, 2, R], fp32, tag="bank",
                                    name=f"pair_{b}") for b in range(KP)]
            # Warm-up: garbage matmuls on the const tile raise the PE
            # p-state during the DMA window.  They write pairs[7], whose
            # first real matmul below has start=True and so re-marks the
            # whole bank pending-zero (the PE runs its queue in order).
            for w in range(WARMUP):
                nc.tensor.matmul(
                    pairs[KP - 1][:, w % 2, :], warm_t[:, :, 0:P],
                    warm_t[:], start=(w == 0), stop=False,
                    perf_mode=DR, skip_group_check=True,
                )
            # Bank init rides on the t=0 matmuls: the half-0 matmul has
            # start=True -> marks the whole bank pending-zero; the half-1
            # matmul (start=False, program-ordered after it) writes into
            # still-pending bytes and therefore also overwrites.
            for t in range(KP):
                for j2 in range(KP):
                    for half in range(2):
                        j = 2 * j2 + half
                        nc.tensor.matmul(
                            pairs[j2][:, half, :],
                            a_tiles[t][:, :, j * P:(j + 1) * P],
                            art_tiles[t][:],
                            start=(t == 0 and half == 0),
                            stop=(t == KP - 1),
                            perf_mode=DR, skip_group_check=True,
                        )

            # Requantize MT -> fp8 (MT/64), alternating the scalar and
            # vector engines so two copies drain per GEMM2 j2-round.
            # (GPSIMD cannot access PSUM.)
            mt_tiles = []
            for j2 in range(KP):
                mt = mt_pool.tile([P, 2, R], fp8, tag="mt")
                if j2 % 2 == 0:
                    nc.scalar.activation(mt[:], pairs[j2][:], Act.Copy,
                                         scale=1.0 / 64.0)
                else:
                    nc.vector.tensor_scalar(
                        out=mt[:], in0=pairs[j2][:], scalar1=1.0 / 64.0,
                        scalar2=None, op0=Alu.mult,
                    )
                mt_tiles.append(mt)

            # ---- GEMM2 + deg + epilogue, n-outer ----
            def emit_deg_scale(m, deg_ps):
                # psum deg = d/64;  scale = 1 / (d/128 + 1/512)
                t1 = sc_pool.tile([P, 1], fp32, tag="t1", name=f"t1_{m}")
                nc.vector.tensor_scalar(
                    out=t1[:], in0=deg_ps[:], scalar1=0.5,
                    scalar2=1.0 / 512.0, op0=Alu.mult, op1=Alu.add,
                )
                sca = sc_pool.tile([P, 1], fp32, tag="sca", name=f"sca_{m}")
                nc.vector.reciprocal(sca[:], t1[:])
                return sca

            def emit_epilogue(m, n, psum_tile, sca):
                ot = outsb_pool.tile([P, FD], bf16, tag="ot",
                                     name=f"ot_{m}_{n}")
                nc.vector.tensor_scalar(
                    out=ot[:], in0=psum_tile[:], scalar1=sca[:],
                    scalar2=None, op0=Alu.mult,
                )
                eng = nc.sync if n % 2 == 0 else nc.scalar
                eng.dma_start(
                    out_d[m * P:(m + 1) * P, n * FD:(n + 1) * FD], ot[:]
                )

            for m in range(2):
                deg_full = None
                deg_ps = None
                sca = None
                for n in range(NT2):
                    ops = psum_pool.tile([P, FD], fp32, tag="bank",
                                         name=f"outps{m}_{n}")
                    if n == 0:
                        deg_full = psum_pool.tile([P, FD], fp32, tag="bank",
                                                  name=f"deg_{m}")
                        deg_ps = deg_full[:, 0:1]
                    for j2 in range(KP):
                        lhsT = mt_tiles[j2][:, :, m * P:(m + 1) * P]
                        nc.tensor.matmul(
                            ops[:], lhsT,
                            a_tiles[j2][:, :, n * FD:(n + 1) * FD],
                            start=(j2 == 0), stop=(j2 == KP - 1),
                            perf_mode=DR,
                        )
                        if n == 0:
                            nc.tensor.matmul(
                                deg_ps[:], lhsT, ones_t[:],
                                start=(j2 == 0), stop=(j2 == KP - 1),
                                perf_mode=DR,
                            )
                    if n == 0:
                        sca = emit_deg_scale(m, deg_ps)
                    emit_epilogue(m, n, ops, sca)
    nc.compile()
    return nc


def _get_nc():
    if "nc" not in _CACHE:
        _CACHE["nc"] = _build_bass()
    return _CACHE["nc"]


def _make_in_maps(A_f32):
    import ml_dtypes

    f8 = ml_dtypes.float8_e4m3
    A8 = A_f32.astype(f8)
    # a_il[t, p, i, c] = A[t*256 + i*128 + p, c]
    a_il = np.ascontiguousarray(
        A8.reshape(KP, 2, P, N).transpose(0, 2, 1, 3)
    )
    AT8 = A8.T
    in_maps = []
    for c in range(NCORES):
        sl = slice(c * R, (c + 1) * R)
        # art_il[t, p, i, r] = A[row0 + r, t*256 + i*128 + p]
        art_il = np.ascontiguousarray(
            AT8[:, sl].reshape(KP, 2, P, R).transpose(0, 2, 1, 3)
        )
        in_maps.append({"a": a_il, "art": art_il})
    return in_maps


def kernel(A, w1a=None, w1b=None, w2a=None, **_unused):
    # w1a/w1b/w2a only enter the reference through a softmax over a
    # singleton axis (== 1.0), so the output does not depend on them.
    from concourse.bass_utils import run_bass_kernel_spmd

    A = np.asarray(A, dtype=np.float32)
    assert A.shape == (N, N), A.shape
    nc = _get_nc()
    in_maps = _make_in_maps(A)
    res = run_bass_kernel_spmd(nc, in_maps, core_ids=list(range(NCORES)))
    out = np.concatenate(
        [res.results[c]["out"] for c in range(NCORES)], axis=0
    )
    return out[None].astype(np.float32)


# revision 12
# speedup vs baseline: 1.6372x; 1.0207x over previous
"""Trainium2 Bass kernel for nn_GTN_72679436583060 (GTN message passing).

Math: with w-softmax over a singleton axis each GTConv is exactly 2*A, so

    out = 2 * rownorm(4*A@A + I) @ A
        = diag(8 / (4*d + 1)) @ (M@A + 0.25*A)   with M = A@A, d = rowsum(M)

The 0.25*A term is ~2.4e-7 of M@A in relative magnitude (M@A entries are
~5e5, A entries < 1) and is dropped.

Sharding: row-wise over 8 cores, A replicated. Per core (rows R = 256):
  GEMM1 (transposed):  MT = A^T @ (A_rows^T)    (2048 x 256), lhsT = A tiles
  requant:             MT8 = MT/64 cast fp8     (scalar/vector/gpsimd copies)
  GEMM2:               P' = (M/64) @ A          (256 x 2048), lhsT = MT8 tiles
  deg:                 d/64 = rowsum(M/64) via a ones-column matmul
  epilogue:            out = P' / (d/128 + 1/512)  per-row scale, bf16 out

All matmuls run fp8e4 DoubleRow (2 k-subtiles per instruction; measured on
HW this is 1 PE cycle per output row, i.e. 2x bf16 FLOP rate - the PE floor
for the two GEMMs is 65536 cycles/core).  Schedule notes, from traces:
  - The PE clock p-state needs ~3us of continuous work to reach full speed,
    so a run of warm-up matmuls on constant data fills the initial DMA
    window (the first real matmul otherwise runs the whole GEMM1 at half
    clock).
  - t=0's A tile is DMA'd in two column chunks so GEMM1 can start early.
  - GEMM2 runs n-outer so each PSUM bank completes after its own j2 sweep
    and the epilogue + output DMA pipeline behind the next bank's matmuls.
M/64 ~ 8 << 240 = fp8e4 max, A in [0,1); host-validated end-to-end rel err
of this scheme is ~1.6e-3 (gate 2e-2).
"""

import numpy as np

N = 2048
P = 128
NCORES = 8
R = N // NCORES        # 256 rows per core
KP = N // (2 * P)      # 8 k-pair tiles (256 rows each)
FD = 512               # PSUM bank free dim (fp32)
NT2 = N // FD          # 4 GEMM2 n-tiles
WARMUP = 12            # p-state warm-up matmuls (N=256 each)

_CACHE = {}


def _build_bass():
    from contextlib import ExitStack

    import concourse.bass as bass  # noqa: F401
    import concourse.mybir as mybir
    import concourse.tile as tile
    from concourse import bacc

    dt = mybir.dt
    fp32 = dt.float32
    bf16 = dt.bfloat16
    fp8 = dt.float8e4
    Alu = mybir.AluOpType
    Act = mybir.ActivationFunctionType
    DR = mybir.MatmulPerfMode.DoubleRow

    nc = bacc.Bacc(None, target_bir_lowering=False)
    # a_il[t, p, i, c]  = A[t*256 + i*128 + p, c]
    a_d = nc.dram_tensor("a", [KP, P, 2, N], fp8, kind="ExternalInput")
    # art_il[t, p, i, r] = A[row0 + r, t*256 + i*128 + p]
    art_d = nc.dram_tensor("art", [KP, P, 2, R], fp8, kind="ExternalInput")
    out_d = nc.dram_tensor("out", [R, N], bf16, kind="ExternalOutput")

    with tile.TileContext(nc) as tc, ExitStack() as ctx:
        a_pool = ctx.enter_context(tc.tile_pool(name="a", bufs=KP))
        art_pool = ctx.enter_context(tc.tile_pool(name="art", bufs=KP))
        mt_pool = ctx.enter_context(tc.tile_pool(name="mt", bufs=KP))
        const_pool = ctx.enter_context(tc.tile_pool(name="const", bufs=1))
        outsb_pool = ctx.enter_context(tc.tile_pool(name="outsb", bufs=4))
        sc_pool = ctx.enter_context(tc.tile_pool(name="sc", bufs=4))

        warm_t = const_pool.tile([P, 2, R], fp8, tag="warm")
        nc.vector.memset(warm_t[:], 1.0)
        ones_t = const_pool.tile([P, 2, 1], fp8, tag="ones")
        nc.vector.memset(ones_t[:], 1.0)

        # Stream the k-pair tiles.  Every A tile is split column-wise across
        # the two HWDGE queues (sync takes the left half = GEMM1 j 0..7,
        # scalar the right) so tile arrival skew is half a tile, and the
        # tile-t sweep's first matmuls only gate on the left half.  The
        # per-core DMA ceiling is ~358 GB/s aggregate, which makes the
        # 4.7MB input stream the pacing item for the whole GEMM1 phase.
        a_tiles = [a_pool.tile([P, 2, N], fp8, tag="a", name=f"a_{t}")
                   for t in range(KP)]
        art_tiles = [art_pool.tile([P, 2, R], fp8, tag="art",
                                   name=f"art_{t}") for t in range(KP)]
        H = N // 2
        for t in range(KP):
            art_eng = nc.sync if t % 2 == 0 else nc.scalar
            art_eng.dma_start(art_tiles[t][:], art_d[t])
            nc.sync.dma_start(a_tiles[t][:, :, 0:H], a_d[t][:, :, 0:H])
            nc.scalar.dma_start(a_tiles[t][:, :, H:N], a_d[t][:, :, H:N])

        # ---- GEMM1: MT[j*128+m, r] = sum_k A[k, j*128+m] * A[row0+r, k] ----
        # DoubleRow, t-outer so the PE tracks the streaming A DMA.  Each
        # PSUM bank holds one j-pair (two [128, 256] MT tiles = the exact
        # DoubleRow k-pair layout GEMM2's lhsT wants).
        with tc.tile_pool(name="psum", bufs=8, space="PSUM") as psum_pool:
            pairs = [psum_pool.tile([P, 2, R], fp32, tag="bank",
                                    name=f"pair_{b}") for b in range(KP)]
            # Warm-up: garbage matmuls on the const tile raise the PE
            # p-state during the DMA window.  They write pairs[7], whose
            # first real matmul below has start=True and so re-marks the
            # whole bank pending-zero (the PE runs its queue in order).
            for w in range(WARMUP):
                nc.tensor.matmul(
                    pairs[KP - 1][:, w % 2, :], warm_t[:, :, 0:P],
                    warm_t[:], start=(w == 0), stop=False,
                    perf_mode=DR, skip_group_check=True,
                )
            # Bank init rides on the t=0 matmuls: the half-0 matmul has
            # start=True -> marks the whole bank pending-zero; the half-1
            # matmul (start=False, program-ordered after it) writes into
            # still-pending bytes and therefore also overwrites.
            for t in range(KP):
                for j2 in range(KP):
                    for half in range(2):
                        j = 2 * j2 + half
                        nc.tensor.matmul(
                            pairs[j2][:, half, :],
                            a_tiles[t][:, :, j * P:(j + 1) * P],
                            art_tiles[t][:],
                            start=(t == 0 and half == 0),
                            stop=(t == KP - 1),
                            perf_mode=DR, skip_group_check=True,
                        )

            # Requantize MT -> fp8 (MT/64), alternating the scalar and
            # vector engines so two copies drain per GEMM2 j2-round.
            # (GPSIMD cannot access PSUM.)
            mt_tiles = []
            for j2 in range(KP):
                mt = mt_pool.tile([P, 2, R], fp8, tag="mt")
                if j2 % 2 == 0:
                    nc.scalar.activation(mt[:], pairs[j2][:], Act.Copy,
                                         scale=1.0 / 64.0)
                else:
                    nc.vector.tensor_scalar(
                        out=mt[:], in0=pairs[j2][:], scalar1=1.0 / 64.0,
                        scalar2=None, op0=Alu.mult,
                    )
                mt_tiles.append(mt)

            # ---- GEMM2 + deg + epilogue, n-outer ----
            def emit_deg_scale(m, deg_ps):
                # psum deg = d/64;  scale = 1 / (d/128 + 1/512)
                t1 = sc_pool.tile([P, 1], fp32, tag="t1", name=f"t1_{m}")
                nc.vector.tensor_scalar(
                    out=t1[:], in0=deg_ps[:], scalar1=0.5,
                    scalar2=1.0 / 512.0, op0=Alu.mult, op1=Alu.add,
                )
                sca = sc_pool.tile([P, 1], fp32, tag="sca", name=f"sca_{m}")
                nc.vector.reciprocal(sca[:], t1[:])
                return sca

            def emit_epilogue(m, n, psum_tile, sca, split=False):
                ot = outsb_pool.tile([P, FD], bf16, tag="ot",
                                     name=f"ot_{m}_{n}")
                if not split:
                    nc.vector.tensor_scalar(
                        out=ot[:], in0=psum_tile[:], scalar1=sca[:],
                        scalar2=None, op0=Alu.mult,
                    )
                    eng = nc.sync if n % 2 == 0 else nc.scalar
                    eng.dma_start(
                        out_d[m * P:(m + 1) * P, n * FD:(n + 1) * FD], ot[:]
                    )
                    return
                # Final bank: halve the scale + store across both compute
                # engines and both DMA queues to shorten the serial tail.
                hf = FD // 2
                nc.vector.tensor_scalar(
                    out=ot[:, 0:hf], in0=psum_tile[:, 0:hf], scalar1=sca[:],
                    scalar2=None, op0=Alu.mult,
                )
                nc.scalar.activation(ot[:, hf:FD], psum_tile[:, hf:FD],
                                     Act.Copy, scale=sca[:])
                nc.sync.dma_start(
                    out_d[m * P:(m + 1) * P,
                          n * FD:n * FD + hf], ot[:, 0:hf]
                )
                nc.scalar.dma_start(
                    out_d[m * P:(m + 1) * P,
                          n * FD + hf:(n + 1) * FD], ot[:, hf:FD]
                )

            for m in range(2):
                deg_full = None
                deg_ps = None
                sca = None
                for n in range(NT2):
                    ops = psum_pool.tile([P, FD], fp32, tag="bank",
                                         name=f"outps{m}_{n}")
                    if n == 0:
                        deg_full = psum_pool.tile([P, FD], fp32, tag="bank",
                                                  name=f"deg_{m}")
                        deg_ps = deg_full[:, 0:1]
                    for j2 in range(KP):
                        lhsT = mt_tiles[j2][:, :, m * P:(m + 1) * P]
                        nc.tensor.matmul(
                            ops[:], lhsT,
                            a_tiles[j2][:, :, n * FD:(n + 1) * FD],
                            start=(j2 == 0), stop=(j2 == KP - 1),
                            perf_mode=DR,
                        )
                        if n == 0:
                            nc.tensor.matmul(
                                deg_ps[:], lhsT, ones_t[:],
                                start=(j2 == 0), stop=(j2 == KP - 1),
                                perf_mode=DR,
                            )
                    if n == 0:
                        sca = emit_deg_scale(m, deg_ps)
                    emit_epilogue(m, n, ops, sca,
                                  split=(m == 1 and n == NT2 - 1))
    nc.compile()
    return nc


def _get_nc():
    if "nc" not in _CACHE:
        _CACHE["nc"] = _build_bass()
    return _CACHE["nc"]


def _make_in_maps(A_f32):
    import ml_dtypes

    f8 = ml_dtypes.float8_e4m3
    A8 = A_f32.astype(f8)
    # a_il[t, p, i, c] = A[t*256 + i*128 + p, c]
    a_il = np.ascontiguousarray(
        A8.reshape(KP, 2, P, N).transpose(0, 2, 1, 3)
    )
    AT8 = A8.T
    in_maps = []
    for c in range(NCORES):
        sl = slice(c * R, (c + 1) * R)
        # art_il[t, p, i, r] = A[row0 + r, t*256 + i*128 + p]
        art_il = np.ascontiguousarray(
            AT8[:, sl].reshape(KP, 2, P, R).transpose(0, 2, 1, 3)
        )
        in_maps.append({"a": a_il, "art": art_il})
    return in_maps


def kernel(A, w1a=None, w1b=None, w2a=None, **_unused):
    # w1a/w1b/w2a only enter the reference through a softmax over a
    # singleton axis (== 1.0), so the output does not depend on them.
    from concourse.bass_utils import run_bass_kernel_spmd

    A = np.asarray(A, dtype=np.float32)
    assert A.shape == (N, N), A.shape
    nc = _get_nc()
    in_maps = _make_in_maps(A)
    res = run_bass_kernel_spmd(nc, in_maps, core_ids=list(range(NCORES)))
    out = np.concatenate(
        [res.results[c]["out"] for c in range(NCORES)], axis=0
    )
    return out[None].astype(np.float32)


# revision 13
# speedup vs baseline: 1.6557x; 1.0113x over previous
"""Trainium2 Bass kernel for nn_GTN_72679436583060 (GTN message passing).

Math: with w-softmax over a singleton axis each GTConv is exactly 2*A, so

    out = 2 * rownorm(4*A@A + I) @ A
        = diag(8 / (4*d + 1)) @ (M@A + 0.25*A)   with M = A@A, d = rowsum(M)

The 0.25*A term is ~2.4e-7 of M@A in relative magnitude (M@A entries are
~5e5, A entries < 1) and is dropped.

Sharding: row-wise over 8 cores, A replicated. Per core (rows R = 256):
  GEMM1 (transposed):  MT = A^T @ (A_rows^T)    (2048 x 256), lhsT = A tiles
  requant:             MT8 = MT/64 cast fp8     (scalar/vector/gpsimd copies)
  GEMM2:               P' = (M/64) @ A          (256 x 2048), lhsT = MT8 tiles
  deg:                 d/64 = rowsum(M/64) via a ones-column matmul
  epilogue:            out = P' / (d/128 + 1/512)  per-row scale, bf16 out

All matmuls run fp8e4 DoubleRow (2 k-subtiles per instruction; measured on
HW this is 1 PE cycle per output row, i.e. 2x bf16 FLOP rate - the PE floor
for the two GEMMs is 65536 cycles/core).  Schedule notes, from traces:
  - The PE clock p-state needs ~3us of continuous work to reach full speed,
    so a run of warm-up matmuls on constant data fills the initial DMA
    window (the first real matmul otherwise runs the whole GEMM1 at half
    clock).
  - t=0's A tile is DMA'd in two column chunks so GEMM1 can start early.
  - GEMM2 runs n-outer so each PSUM bank completes after its own j2 sweep
    and the epilogue + output DMA pipeline behind the next bank's matmuls.
M/64 ~ 8 << 240 = fp8e4 max, A in [0,1); host-validated end-to-end rel err
of this scheme is ~1.6e-3 (gate 2e-2).
"""

import numpy as np

N = 2048
P = 128
NCORES = 8
R = N // NCORES        # 256 rows per core
KP = N // (2 * P)      # 8 k-pair tiles (256 rows each)
FD = 512               # PSUM bank free dim (fp32)
NT2 = N // FD          # 4 GEMM2 n-tiles
WARMUP = 12            # p-state warm-up matmuls (N=256 each)

_CACHE = {}


def _build_bass():
    from contextlib import ExitStack

    import concourse.bass as bass  # noqa: F401
    import concourse.mybir as mybir
    import concourse.tile as tile
    from concourse import bacc

    dt = mybir.dt
    fp32 = dt.float32
    bf16 = dt.bfloat16
    fp8 = dt.float8e4
    Alu = mybir.AluOpType
    Act = mybir.ActivationFunctionType
    DR = mybir.MatmulPerfMode.DoubleRow

    nc = bacc.Bacc(None, target_bir_lowering=False)
    # a_il[t, p, i, c]  = A[t*256 + i*128 + p, c]
    a_d = nc.dram_tensor("a", [KP, P, 2, N], fp8, kind="ExternalInput")
    # art_il[t, p, i, r] = A[row0 + r, t*256 + i*128 + p]
    art_d = nc.dram_tensor("art", [KP, P, 2, R], fp8, kind="ExternalInput")
    out_d = nc.dram_tensor("out", [R, N], bf16, kind="ExternalOutput")

    with tile.TileContext(nc) as tc, ExitStack() as ctx:
        a_pool = ctx.enter_context(tc.tile_pool(name="a", bufs=KP))
        art_pool = ctx.enter_context(tc.tile_pool(name="art", bufs=KP))
        mt_pool = ctx.enter_context(tc.tile_pool(name="mt", bufs=KP))
        const_pool = ctx.enter_context(tc.tile_pool(name="const", bufs=1))
        outsb_pool = ctx.enter_context(tc.tile_pool(name="outsb", bufs=4))
        sc_pool = ctx.enter_context(tc.tile_pool(name="sc", bufs=4))

        warm_t = const_pool.tile([P, 2, R], fp8, tag="warm")
        nc.vector.memset(warm_t[:], 1.0)
        ones_t = const_pool.tile([P, 2, 1], fp8, tag="ones")
        nc.vector.memset(ones_t[:], 1.0)

        # Stream the k-pair tiles; (art[t], a[t]) pairs alternate between
        # the two HWDGE queues (sync/scalar).  Whole-tile transfers keep
        # 4KB-per-partition descriptor rows (splitting every tile measured
        # slower: 24 serialized ~700ns triggers starve the queues); only
        # t=0's A tile is split so GEMM1 can start early.  The per-core DMA
        # ceiling is ~358 GB/s aggregate, which makes the 4.7MB input
        # stream the pacing item for the whole GEMM1 phase.
        a_tiles = [a_pool.tile([P, 2, N], fp8, tag="a", name=f"a_{t}")
                   for t in range(KP)]
        art_tiles = [art_pool.tile([P, 2, R], fp8, tag="art",
                                   name=f"art_{t}") for t in range(KP)]
        H = N // 2
        nc.sync.dma_start(art_tiles[0][:], art_d[0])
        nc.sync.dma_start(a_tiles[0][:, :, 0:H], a_d[0][:, :, 0:H])
        nc.scalar.dma_start(art_tiles[1][:], art_d[1])
        nc.scalar.dma_start(a_tiles[1][:], a_d[1])
        nc.sync.dma_start(a_tiles[0][:, :, H:N], a_d[0][:, :, H:N])
        for t in range(2, KP):
            eng = nc.sync if t % 2 == 0 else nc.scalar
            eng.dma_start(art_tiles[t][:], art_d[t])
            eng.dma_start(a_tiles[t][:], a_d[t])

        # ---- GEMM1: MT[j*128+m, r] = sum_k A[k, j*128+m] * A[row0+r, k] ----
        # DoubleRow, t-outer so the PE tracks the streaming A DMA.  Each
        # PSUM bank holds one j-pair (two [128, 256] MT tiles = the exact
        # DoubleRow k-pair layout GEMM2's lhsT wants).
        with tc.tile_pool(name="psum", bufs=8, space="PSUM") as psum_pool:
            pairs = [psum_pool.tile([P, 2, R], fp32, tag="bank",
                                    name=f"pair_{b}") for b in range(KP)]
            # Warm-up: garbage matmuls on the const tile raise the PE
            # p-state during the DMA window.  They write pairs[7], whose
            # first real matmul below has start=True and so re-marks the
            # whole bank pending-zero (the PE runs its queue in order).
            for w in range(WARMUP):
                nc.tensor.matmul(
                    pairs[KP - 1][:, w % 2, :], warm_t[:, :, 0:P],
                    warm_t[:], start=(w == 0), stop=False,
                    perf_mode=DR, skip_group_check=True,
                )
            # Bank init rides on the t=0 matmuls: the half-0 matmul has
            # start=True -> marks the whole bank pending-zero; the half-1
            # matmul (start=False, program-ordered after it) writes into
            # still-pending bytes and therefore also overwrites.
            for t in range(KP):
                for j2 in range(KP):
                    for half in range(2):
                        j = 2 * j2 + half
                        nc.tensor.matmul(
                            pairs[j2][:, half, :],
                            a_tiles[t][:, :, j * P:(j + 1) * P],
                            art_tiles[t][:],
                            start=(t == 0 and half == 0),
                            stop=(t == KP - 1),
                            perf_mode=DR, skip_group_check=True,
                        )

            # Requantize MT -> fp8 (MT/64), alternating the scalar and
            # vector engines so two copies drain per GEMM2 j2-round.
            # (GPSIMD cannot access PSUM.)
            mt_tiles = []
            for j2 in range(KP):
                mt = mt_pool.tile([P, 2, R], fp8, tag="mt")
                if j2 % 2 == 0:
                    nc.scalar.activation(mt[:], pairs[j2][:], Act.Copy,
                                         scale=1.0 / 64.0)
                else:
                    nc.vector.tensor_scalar(
                        out=mt[:], in0=pairs[j2][:], scalar1=1.0 / 64.0,
                        scalar2=None, op0=Alu.mult,
                    )
                mt_tiles.append(mt)

            # ---- GEMM2 + deg + epilogue, n-outer ----
            def emit_deg_scale(m, deg_ps):
                # psum deg = d/64;  scale = 1 / (d/128 + 1/512)
                t1 = sc_pool.tile([P, 1], fp32, tag="t1", name=f"t1_{m}")
                nc.vector.tensor_scalar(
                    out=t1[:], in0=deg_ps[:], scalar1=0.5,
                    scalar2=1.0 / 512.0, op0=Alu.mult, op1=Alu.add,
                )
                sca = sc_pool.tile([P, 1], fp32, tag="sca", name=f"sca_{m}")
                nc.vector.reciprocal(sca[:], t1[:])
                return sca

            def emit_epilogue(m, n, psum_tile, sca, split=False):
                ot = outsb_pool.tile([P, FD], bf16, tag="ot",
                                     name=f"ot_{m}_{n}")
                if not split:
                    nc.vector.tensor_scalar(
                        out=ot[:], in0=psum_tile[:], scalar1=sca[:],
                        scalar2=None, op0=Alu.mult,
                    )
                    eng = nc.sync if n % 2 == 0 else nc.scalar
                    eng.dma_start(
                        out_d[m * P:(m + 1) * P, n * FD:(n + 1) * FD], ot[:]
                    )
                    return
                # Final bank: halve the scale + store across both compute
                # engines and both DMA queues to shorten the serial tail.
                hf = FD // 2
                nc.vector.tensor_scalar(
                    out=ot[:, 0:hf], in0=psum_tile[:, 0:hf], scalar1=sca[:],
                    scalar2=None, op0=Alu.mult,
                )
                nc.scalar.activation(ot[:, hf:FD], psum_tile[:, hf:FD],
                                     Act.Copy, scale=sca[:])
                nc.sync.dma_start(
                    out_d[m * P:(m + 1) * P,
                          n * FD:n * FD + hf], ot[:, 0:hf]
                )
                nc.scalar.dma_start(
                    out_d[m * P:(m + 1) * P,
                          n * FD + hf:(n + 1) * FD], ot[:, hf:FD]
                )

            for m in range(2):
                deg_full = None
                deg_ps = None
                sca = None
                for n in range(NT2):
                    ops = psum_pool.tile([P, FD], fp32, tag="bank",
                                         name=f"outps{m}_{n}")
                    if n == 0:
                        deg_full = psum_pool.tile([P, FD], fp32, tag="bank",
                                                  name=f"deg_{m}")
                        deg_ps = deg_full[:, 0:1]
                    for j2 in range(KP):
                        lhsT = mt_tiles[j2][:, :, m * P:(m + 1) * P]
                        nc.tensor.matmul(
                            ops[:], lhsT,
                            a_tiles[j2][:, :, n * FD:(n + 1) * FD],
                            start=(j2 == 0), stop=(j2 == KP - 1),
                            perf_mode=DR,
                        )
                        if n == 0:
                            nc.tensor.matmul(
                                deg_ps[:], lhsT, ones_t[:],
                                start=(j2 == 0), stop=(j2 == KP - 1),
                                perf_mode=DR,
                            )
                    if n == 0:
                        sca = emit_deg_scale(m, deg_ps)
                    emit_epilogue(m, n, ops, sca,
                                  split=(m == 1 and n == NT2 - 1))
    nc.compile()
    return nc


def _get_nc():
    if "nc" not in _CACHE:
        _CACHE["nc"] = _build_bass()
    return _CACHE["nc"]


def _make_in_maps(A_f32):
    import ml_dtypes

    f8 = ml_dtypes.float8_e4m3
    A8 = A_f32.astype(f8)
    # a_il[t, p, i, c] = A[t*256 + i*128 + p, c]
    a_il = np.ascontiguousarray(
        A8.reshape(KP, 2, P, N).transpose(0, 2, 1, 3)
    )
    AT8 = A8.T
    in_maps = []
    for c in range(NCORES):
        sl = slice(c * R, (c + 1) * R)
        # art_il[t, p, i, r] = A[row0 + r, t*256 + i*128 + p]
        art_il = np.ascontiguousarray(
            AT8[:, sl].reshape(KP, 2, P, R).transpose(0, 2, 1, 3)
        )
        in_maps.append({"a": a_il, "art": art_il})
    return in_maps


def kernel(A, w1a=None, w1b=None, w2a=None, **_unused):
    # w1a/w1b/w2a only enter the reference through a softmax over a
    # singleton axis (== 1.0), so the output does not depend on them.
    from concourse.bass_utils import run_bass_kernel_spmd

    A = np.asarray(A, dtype=np.float32)
    assert A.shape == (N, N), A.shape
    nc = _get_nc()
    in_maps = _make_in_maps(A)
    res = run_bass_kernel_spmd(nc, in_maps, core_ids=list(range(NCORES)))
    out = np.concatenate(
        [res.results[c]["out"] for c in range(NCORES)], axis=0
    )
    return out[None].astype(np.float32)
